# revision 8
# baseline (speedup 1.0000x reference)
"""BilinearPooling kernel for TRN2 (8 NeuronCores, pure data parallel).

Reference math: out[b, k] = mean_j(conv1[b, j]) * conv2[b, k], with
conv1/conv2 flattened to [B, 50176] from [256, 14, 14, 256].

Sharding: batch dim B=256 split across 8 cores -> 32 samples/core.
Per-core layout: the [32, 50176] slice is viewed as [128, 12544] so sample b
occupies partitions 4b..4b+3.  Per-partition sums of c1 feed one fp32 matmul
against a block-diagonal (1/J) matrix that sums each group of 4 partitions
and broadcasts the per-sample mean back to its 4 partitions.  conv2 streams
through SBUF with a per-partition scalar multiply.

Precision: the kernel is HBM-bandwidth bound (both HWDGE queues together
sustain ~420 GB/s per core), so all three big tensors move as bf16 (host
casts inputs, host upcasts the output).  Measured scale-relative error vs
the f32 reference is ~4.3e-3, well inside the 2e-2 gate.  Traffic per core
drops 19.3 MB -> 9.6 MB, so the pipe floor is ~23 us.

Schedule per core: SYNC streams c1 then c2 loads on its HWDGE ring; ACT
owns the other ring (blockdiag load + output stores).  Chunk layout rules
learned from traces: per-partition DMA lines below ~2 KB crawl at 5-15x
under line rate (so every chunk keeps lines >= 3 KB), transfers ~1 MB are
needed for full rate (so middle chunks are 4608 elems = 1.18 MB), and a
chunk's completion semaphore fires ~2 us after its last byte (HBM receipt
round trip), which puts the LAST c1 chunk's fold on the critical path to
the scale — hence c1 ends with a short 1792 chunk.

The c1 chunk folds run at ~1 elem/cycle/partition, slower than arrival,
so each chunk's fold is SPLIT between DVE (tensor_scalar with accum_out)
and ACT (activation Copy with accum_out), which together outpace the DMA.
The chunk sums land in `partials`; the combine reduce is semaphore-gated
on both engines' folds because the accumulator writeback completes after
the instruction's main phase (reading partials from the next instruction
slot races it).  PE does the tiny block-diag matmul; ACT copies the f32
PSUM scale into SBUF (the DVE scalar operand must be f32) and issues the
store triggers, each gated on its mul's semaphore (one attached wait is
the DGE ISA limit; the wait blocks only ACT's own sequencer, which is
otherwise idle by then).  Mul/store slices are ~1536 elems so stores join
the pipe as soon as the scale exists.  No final wait on the store
semaphore: the per-engine epilogue drains the ring.

DMA completion semaphores are PER CHUNK (wait >= 16): a single cumulative
semaphore is racy because the 16 SDMA lanes increment independently and a
fast lane can run several chunks ahead of a slow one.
"""

from contextlib import ExitStack

import ml_dtypes
import numpy as np

import concourse.bass as bass
import concourse.mybir as mybir
from concourse.bass_utils import run_bass_kernel_spmd

B = 256          # full batch
J = 50176        # flattened feature dim (14*14*256)
NCORES = 8
BPC = B // NCORES          # 32 samples per core
P = 128                    # SBUF partitions
RPS = P // BPC             # 4 partition-rows per sample
F = J // RPS               # 12544 free elems per partition (bf16)

# c1 load chunks: warmup chunk (the ring's first transfer runs ~2x slow),
# two 1.18 MB chunks, then a short tail so the last fold is quick.
C1_SIZES = [1536, 4608, 4608, 1792]
C1_OFFS = [sum(C1_SIZES[:i]) for i in range(len(C1_SIZES))]
assert sum(C1_SIZES) == F
# DVE/ACT fold split per c1 chunk (DVE elems; ACT takes the rest).  Chunk 0
# is DVE-only (ACT is still loading its activation table that early).
# Balance: sz_dve/0.96 + 0.27us == sz_act/1.2 + 0.55us.
DVE_SPLIT = {4608: 2200, 1792: 856}
# c2 load chunks, same line-size rules; tail chunks shortish so the last
# mul + store are quick.
C2_SIZES = [4608, 4608, 1792, 1536]
C2_OFFS = [sum(C2_SIZES[:i]) for i in range(len(C2_SIZES))]
assert sum(C2_SIZES) == F
# mul/store slices: (load_chunk_idx, offset, size), all >= 1536 elems.
C2_SLICES = []
for i, (off, sz) in enumerate(zip(C2_OFFS, C2_SIZES)):
    n = max(1, sz // 1536)
    step = sz // n
    for k in range(n):
        s = step if k < n - 1 else sz - step * (n - 1)
        C2_SLICES.append((i, off + k * step, s))
assert sum(s for _, _, s in C2_SLICES) == F

FP32 = mybir.dt.float32
BF16 = mybir.dt.bfloat16
AX = mybir.AxisListType.X
ADD = mybir.AluOpType.add
MULT = mybir.AluOpType.mult
COPY = mybir.ActivationFunctionType.Copy

# Stashed by kernel() for test harnesses that want timing/trace info.
LAST_RESULTS = None


def _build_nc():
    nc = bass.Bass(monotonic_sem_count=0)
    c1 = nc.dram_tensor("conv1", [P, F], BF16, kind="ExternalInput")
    c2 = nc.dram_tensor("conv2", [P, F], BF16, kind="ExternalInput")
    bd = nc.dram_tensor("blockdiag", [P, P], FP32, kind="ExternalInput")
    out = nc.dram_tensor("out", [P, F], BF16, kind="ExternalOutput")

    ndve = len(C1_SIZES)
    nact = len(C1_SIZES) - 1
    nfold = ndve + nact

    with ExitStack() as ctx:
        ec = ctx.enter_context
        c1t = [
            ec(nc.sbuf_tensor(f"c1t{i}", [P, sz], BF16))
            for i, sz in enumerate(C1_SIZES)
        ]
        c2t = [
            ec(nc.sbuf_tensor(f"c2t{i}", [P, sz], BF16))
            for i, sz in enumerate(C2_SIZES)
        ]
        ot = [
            ec(nc.sbuf_tensor(f"ot{i}", [P, s], BF16))
            for i, (_, _, s) in enumerate(C2_SLICES)
        ]
        scr_v = ec(nc.sbuf_tensor("scr_v", [P, max(C1_SIZES)], BF16))
        scr_a = ec(nc.sbuf_tensor("scr_a", [P, max(C1_SIZES)], BF16))
        bdt = ec(nc.sbuf_tensor("bdt", [P, P], FP32))
        partials = ec(nc.sbuf_tensor("partials", [P, nfold], FP32))
        sums = ec(nc.sbuf_tensor("sums", [P, 1], FP32))
        scale_f = ec(nc.sbuf_tensor("scale_f", [P, 1], FP32))
        pscale = ec(nc.psum_tensor("pscale", [P, 1], FP32))

        bds = ec(nc.semaphore("bds"))
        c1s = [ec(nc.semaphore(f"c1s{i}")) for i in range(len(C1_SIZES))]
        c2s = [ec(nc.semaphore(f"c2s{i}")) for i in range(len(C2_SIZES))]
        fdv = ec(nc.semaphore("fdv"))
        fda = ec(nc.semaphore("fda"))
        red = ec(nc.semaphore("red"))
        mms = ec(nc.semaphore("mms"))
        sc = ec(nc.semaphore("sc"))
        muls = ec(nc.semaphore("muls"))
        sts = ec(nc.semaphore("sts"))

        # No nc.Block: instructions are emitted straight into the main basic
        # block (each tagged with its engine), which skips the Block entry
        # branches and the exit all-engine barrier.
        nc.scalar.dma_start(bdt[:], bd[:]).then_inc(bds, 16)
        for i, (off, sz) in enumerate(zip(C1_OFFS, C1_SIZES)):
            nc.sync.dma_start(c1t[i][:], c1[:, off : off + sz]).then_inc(c1s[i], 16)
        for i, (off, sz) in enumerate(zip(C2_OFFS, C2_SIZES)):
            nc.sync.dma_start(c2t[i][:], c2[:, off : off + sz]).then_inc(c2s[i], 16)

        # c1 chunk folds.  DVE: tensor_scalar(x*1.0, reduce-add accum_out);
        # the dummy elementwise result goes to a scratch tile nobody reads.
        # ACT: activation Copy with accum_out on the back part of the chunk.
        nc.vector.wait_ge(c1s[0], 16)
        nc.vector.tensor_scalar(
            scr_v[:, 0 : C1_SIZES[0]],
            c1t[0][:],
            1.0,
            None,
            op0=MULT,
            op1=ADD,
            accum_out=partials[:, 0:1],
        ).then_inc(fdv, 1)
        for i in range(1, len(C1_SIZES)):
            sz = C1_SIZES[i]
            dp = DVE_SPLIT[sz]
            nc.vector.wait_ge(c1s[i], 16)
            nc.vector.tensor_scalar(
                scr_v[:, 0:dp],
                c1t[i][:, 0:dp],
                1.0,
                None,
                op0=MULT,
                op1=ADD,
                accum_out=partials[:, i : i + 1],
            ).then_inc(fdv, 1)
            nc.scalar.wait_ge(c1s[i], 16)
            nc.scalar.activation(
                scr_a[:, 0 : sz - dp],
                c1t[i][:, dp:sz],
                COPY,
                accum_out=partials[:, ndve + i - 1 : ndve + i],
            ).then_inc(fda, 1)

        # The accumulator writeback lands after the instruction's main phase;
        # gate the combine on both engines' fold semaphores (which fire at
        # full completion) instead of relying on program order.
        nc.vector.wait_ge(fdv, ndve)
        nc.vector.wait_ge(fda, nact)
        nc.vector.reduce_sum(sums[:], partials[:], axis=AX).then_inc(red, 1)

        nc.tensor.wait_ge(bds, 16)
        nc.tensor.wait_ge(red, 1)
        nc.tensor.matmul(
            pscale[:], bdt[:], sums[:], start=True, stop=True
        ).then_inc(mms, 1)

        nc.scalar.wait_ge(mms, 1)
        nc.scalar.copy(scale_f[:], pscale[:, 0:1]).then_inc(sc, 1)

        nc.vector.wait_ge(sc, 1)
        for j, (ci, off, sz) in enumerate(C2_SLICES):
            nc.vector.wait_ge(c2s[ci], 16)
            rel = off - C2_OFFS[ci]
            nc.vector.tensor_scalar_mul(
                ot[j][:], c2t[ci][:, rel : rel + sz], scale_f[:, 0:1]
            ).then_inc(muls, 1)

        for j, (ci, off, sz) in enumerate(C2_SLICES):
            # One attached wait per DGE DMA (the ISA limit); the wait blocks
            # only ACT's sequencer, idle by this point.  muls is incremented
            # by one in-order engine (DVE), so the cumulative threshold is
            # exact.
            nc.scalar.dma_start(out[:, off : off + sz], ot[j][:])._wait_ge(
                muls, j + 1
            ).then_inc(sts, 16)
        # No final wait on sts: the per-engine epilogue (drain + sem chain +
        # NOTIFY) already runs after the ACT stream ends and covers the
        # in-flight stores; an explicit wait would serialize the epilogue
        # after them and lengthen the measured window.

    # Drop SP's wait-half of the framework's entry barrier (its preceding
    # DRAIN still increments the gather sem, so the leader and the other
    # engines synchronize as before).  SP then issues the first load trigger
    # right after its own preamble instead of waiting for the straggler
    # engine.  Safe by timing: the earliest DMA semaphore increment lands
    # well after every engine's sem-zeroing chain ends.
    mb = nc.main_func.blocks[0]
    for ins in list(mb.instructions):
        if (ins.name or "").startswith("barrier_SP_"):
            mb.instructions.remove(ins)
            break

    return nc


def kernel(conv1, conv2, _trace=False):
    global LAST_RESULTS
    c1 = np.asarray(conv1, dtype=np.float32).reshape(B, J)
    c2 = np.asarray(conv2, dtype=np.float32).reshape(B, J)
    c1_bf = c1.astype(ml_dtypes.bfloat16)
    c2_bf = c2.astype(ml_dtypes.bfloat16)

    # blockdiag[p, m] = 1/J if p//RPS == m//RPS else 0
    bd = (
        np.kron(np.eye(BPC, dtype=np.float32), np.ones((RPS, RPS), dtype=np.float32))
        / np.float32(J)
    ).astype(np.float32)

    in_maps = []
    for i in range(NCORES):
        sl = slice(i * BPC, (i + 1) * BPC)
        in_maps.append(
            {
                "conv1": np.ascontiguousarray(c1_bf[sl].reshape(P, F)),
                "conv2": np.ascontiguousarray(c2_bf[sl].reshape(P, F)),
                "blockdiag": bd,
            }
        )

    nc = _build_nc()
    res = run_bass_kernel_spmd(nc, in_maps, list(range(NCORES)), trace=bool(_trace))
    LAST_RESULTS = res
    out = np.concatenate(
        [
            np.asarray(res.results[i]["out"]).reshape(BPC, J)
            for i in range(NCORES)
        ],
        axis=0,
    ).astype(np.float32)
    return out


# revision 9
# speedup vs baseline: 1.1079x; 1.1079x over previous
"""BilinearPooling kernel for TRN2 (8 NeuronCores, pure data parallel).

Reference math: out[b, k] = mean_j(conv1[b, j]) * conv2[b, k], with
conv1/conv2 flattened to [B, 50176] from [256, 14, 14, 256].

Sharding: batch dim B=256 split across 8 cores -> 32 samples/core.
Per-core layout: the [32, 50176] slice is viewed as [128, 12544] so sample b
occupies partitions 4b..4b+3.  Per-partition sums of c1 feed one fp32 matmul
against a block-diagonal (1/J) matrix that sums each group of 4 partitions
and broadcasts the per-sample mean back to its 4 partitions.  conv2 streams
through SBUF with a per-partition scalar multiply.

Precision: the kernel is HBM-bandwidth bound (both HWDGE queues together
sustain ~420 GB/s per core), so all three big tensors move as bf16 (host
casts inputs, host upcasts the output).  Measured scale-relative error vs
the f32 reference is ~4.3e-3, well inside the 2e-2 gate.  Traffic per core
drops 19.3 MB -> 9.6 MB, so the pipe floor is ~23 us.

Schedule per core: SYNC streams c1 then c2 loads on its HWDGE ring; ACT
owns the other ring (blockdiag load + output stores).  Chunk layout rules
learned from traces: per-partition DMA lines below ~2 KB crawl at 5-15x
under line rate (so every chunk keeps lines >= 3 KB), transfers ~1 MB are
needed for full rate (so middle chunks are 4608 elems = 1.18 MB), and a
chunk's completion semaphore fires ~2 us after its last byte (HBM receipt
round trip), which puts the LAST c1 chunk's fold on the critical path to
the scale — hence c1 ends with a short 1792 chunk.

The c1 chunk folds run at ~1 elem/cycle/partition, slower than arrival,
so each chunk's fold is SPLIT between DVE (tensor_scalar with accum_out)
and ACT (activation Copy with accum_out), which together outpace the DMA.
The chunk sums land in `partials`; the combine reduce is semaphore-gated
on both engines' folds because the accumulator writeback completes after
the instruction's main phase (reading partials from the next instruction
slot races it).  PE does the tiny block-diag matmul; ACT copies the f32
PSUM scale into SBUF (the DVE scalar operand must be f32) and issues the
store triggers, each gated on its mul's semaphore (one attached wait is
the DGE ISA limit; the wait blocks only ACT's own sequencer, which is
otherwise idle by then).  Mul/store slices are ~1536 elems so stores join
the pipe as soon as the scale exists.  No final wait on the store
semaphore: the per-engine epilogue drains the ring.

DMA completion semaphores are PER CHUNK (wait >= 16): a single cumulative
semaphore is racy because the 16 SDMA lanes increment independently and a
fast lane can run several chunks ahead of a slow one.
"""

from contextlib import ExitStack

import ml_dtypes
import numpy as np

import concourse.bass as bass
import concourse.mybir as mybir
from concourse.bass_utils import run_bass_kernel_spmd

B = 256          # full batch
J = 50176        # flattened feature dim (14*14*256)
NCORES = 8
BPC = B // NCORES          # 32 samples per core
P = 128                    # SBUF partitions
RPS = P // BPC             # 4 partition-rows per sample
F = J // RPS               # 12544 free elems per partition (bf16)

# c1 load chunks: warmup chunk (the ring's first transfer runs ~2x slow),
# two 1.18 MB chunks, then a short tail so the last fold is quick.
C1_SIZES = [1536, 4608, 4608, 1792]
C1_OFFS = [sum(C1_SIZES[:i]) for i in range(len(C1_SIZES))]
assert sum(C1_SIZES) == F
# DVE/ACT fold split per c1 chunk (DVE elems; ACT takes the rest).  Chunk 0
# is DVE-only (ACT is still loading its activation table that early).
# Balance: sz_dve/0.96 + 0.27us == sz_act/1.2 + 0.55us.
DVE_SPLIT = {4608: 2200, 1792: 856}
# c2 load chunks, same line-size rules; tail chunks shortish so the last
# mul + store are quick.
C2_SIZES = [4608, 4608, 1792, 1536]
C2_OFFS = [sum(C2_SIZES[:i]) for i in range(len(C2_SIZES))]
assert sum(C2_SIZES) == F
# mul/store slices: (load_chunk_idx, offset, size), all >= 1536 elems.
# Store-trigger execution costs ~0.65us of sequencer time apiece (the
# attached wait blocks the issuing engine), so slices are few and their
# triggers alternate between SYNC (idle once loads are triggered) and ACT.
C2_SLICES = []
for i, (off, sz) in enumerate(zip(C2_OFFS, C2_SIZES)):
    n = max(1, sz // 2304)
    step = sz // n
    for k in range(n):
        s = step if k < n - 1 else sz - step * (n - 1)
        C2_SLICES.append((i, off + k * step, s))
assert sum(s for _, _, s in C2_SLICES) == F

FP32 = mybir.dt.float32
BF16 = mybir.dt.bfloat16
AX = mybir.AxisListType.X
ADD = mybir.AluOpType.add
MULT = mybir.AluOpType.mult
COPY = mybir.ActivationFunctionType.Copy

# Stashed by kernel() for test harnesses that want timing/trace info.
LAST_RESULTS = None


def _build_nc():
    nc = bass.Bass(monotonic_sem_count=0)
    c1 = nc.dram_tensor("conv1", [P, F], BF16, kind="ExternalInput")
    c2 = nc.dram_tensor("conv2", [P, F], BF16, kind="ExternalInput")
    bd = nc.dram_tensor("blockdiag", [P, P], FP32, kind="ExternalInput")
    out = nc.dram_tensor("out", [P, F], BF16, kind="ExternalOutput")

    ndve = len(C1_SIZES)
    nact = len(C1_SIZES) - 1
    nfold = ndve + nact

    with ExitStack() as ctx:
        ec = ctx.enter_context
        c1t = [
            ec(nc.sbuf_tensor(f"c1t{i}", [P, sz], BF16))
            for i, sz in enumerate(C1_SIZES)
        ]
        c2t = [
            ec(nc.sbuf_tensor(f"c2t{i}", [P, sz], BF16))
            for i, sz in enumerate(C2_SIZES)
        ]
        ot = [
            ec(nc.sbuf_tensor(f"ot{i}", [P, s], BF16))
            for i, (_, _, s) in enumerate(C2_SLICES)
        ]
        scr_v = ec(nc.sbuf_tensor("scr_v", [P, max(C1_SIZES)], BF16))
        scr_a = ec(nc.sbuf_tensor("scr_a", [P, max(C1_SIZES)], BF16))
        bdt = ec(nc.sbuf_tensor("bdt", [P, P], FP32))
        partials = ec(nc.sbuf_tensor("partials", [P, nfold], FP32))
        sums = ec(nc.sbuf_tensor("sums", [P, 1], FP32))
        scale_f = ec(nc.sbuf_tensor("scale_f", [P, 1], FP32))
        pscale = ec(nc.psum_tensor("pscale", [P, 1], FP32))

        bds = ec(nc.semaphore("bds"))
        c1s = [ec(nc.semaphore(f"c1s{i}")) for i in range(len(C1_SIZES))]
        c2s = [ec(nc.semaphore(f"c2s{i}")) for i in range(len(C2_SIZES))]
        fdv = ec(nc.semaphore("fdv"))
        fda = ec(nc.semaphore("fda"))
        red = ec(nc.semaphore("red"))
        mms = ec(nc.semaphore("mms"))
        sc = ec(nc.semaphore("sc"))
        muls = ec(nc.semaphore("muls"))
        sts = ec(nc.semaphore("sts"))

        # No nc.Block: instructions are emitted straight into the main basic
        # block (each tagged with its engine), which skips the Block entry
        # branches and the exit all-engine barrier.
        nc.scalar.dma_start(bdt[:], bd[:]).then_inc(bds, 16)
        for i, (off, sz) in enumerate(zip(C1_OFFS, C1_SIZES)):
            nc.sync.dma_start(c1t[i][:], c1[:, off : off + sz]).then_inc(c1s[i], 16)
        for i, (off, sz) in enumerate(zip(C2_OFFS, C2_SIZES)):
            nc.sync.dma_start(c2t[i][:], c2[:, off : off + sz]).then_inc(c2s[i], 16)

        # c1 chunk folds.  DVE: tensor_scalar(x*1.0, reduce-add accum_out);
        # the dummy elementwise result goes to a scratch tile nobody reads.
        # ACT: activation Copy with accum_out on the back part of the chunk.
        nc.vector.wait_ge(c1s[0], 16)
        nc.vector.tensor_scalar(
            scr_v[:, 0 : C1_SIZES[0]],
            c1t[0][:],
            1.0,
            None,
            op0=MULT,
            op1=ADD,
            accum_out=partials[:, 0:1],
        ).then_inc(fdv, 1)
        for i in range(1, len(C1_SIZES)):
            sz = C1_SIZES[i]
            dp = DVE_SPLIT[sz]
            nc.vector.wait_ge(c1s[i], 16)
            nc.vector.tensor_scalar(
                scr_v[:, 0:dp],
                c1t[i][:, 0:dp],
                1.0,
                None,
                op0=MULT,
                op1=ADD,
                accum_out=partials[:, i : i + 1],
            ).then_inc(fdv, 1)
            nc.scalar.wait_ge(c1s[i], 16)
            nc.scalar.activation(
                scr_a[:, 0 : sz - dp],
                c1t[i][:, dp:sz],
                COPY,
                accum_out=partials[:, ndve + i - 1 : ndve + i],
            ).then_inc(fda, 1)

        # The accumulator writeback lands after the instruction's main phase;
        # gate the combine on both engines' fold semaphores (which fire at
        # full completion) instead of relying on program order.
        nc.vector.wait_ge(fdv, ndve)
        nc.vector.wait_ge(fda, nact)
        nc.vector.reduce_sum(sums[:], partials[:], axis=AX).then_inc(red, 1)

        nc.tensor.wait_ge(bds, 16)
        nc.tensor.wait_ge(red, 1)
        nc.tensor.matmul(
            pscale[:], bdt[:], sums[:], start=True, stop=True
        ).then_inc(mms, 1)

        nc.scalar.wait_ge(mms, 1)
        nc.scalar.copy(scale_f[:], pscale[:, 0:1]).then_inc(sc, 1)

        nc.vector.wait_ge(sc, 1)
        for j, (ci, off, sz) in enumerate(C2_SLICES):
            nc.vector.wait_ge(c2s[ci], 16)
            rel = off - C2_OFFS[ci]
            nc.vector.tensor_scalar_mul(
                ot[j][:], c2t[ci][:, rel : rel + sz], scale_f[:, 0:1]
            ).then_inc(muls, 1)

        for j, (ci, off, sz) in enumerate(C2_SLICES):
            # One attached wait per DGE DMA (the ISA limit); the wait blocks
            # the issuing engine's sequencer, so triggers alternate between
            # the two HWDGE engines to halve the serial trigger chain.  muls
            # is incremented by one in-order engine (DVE), so the cumulative
            # threshold is exact.
            eng = nc.scalar if j % 2 == 0 else nc.sync
            eng.dma_start(out[:, off : off + sz], ot[j][:])._wait_ge(
                muls, j + 1
            ).then_inc(sts, 16)
        # No final wait on sts: the per-engine epilogue (drain + sem chain +
        # NOTIFY) already runs after the ACT stream ends and covers the
        # in-flight stores; an explicit wait would serialize the epilogue
        # after them and lengthen the measured window.

    # Drop SP's wait-half of the framework's entry barrier (its preceding
    # DRAIN still increments the gather sem, so the leader and the other
    # engines synchronize as before).  SP then issues the first load trigger
    # right after its own preamble instead of waiting for the straggler
    # engine.  Safe by timing: the earliest DMA semaphore increment lands
    # well after every engine's sem-zeroing chain ends.
    mb = nc.main_func.blocks[0]
    for ins in list(mb.instructions):
        if (ins.name or "").startswith("barrier_SP_"):
            mb.instructions.remove(ins)
            break

    return nc


def kernel(conv1, conv2, _trace=False):
    global LAST_RESULTS
    c1 = np.asarray(conv1, dtype=np.float32).reshape(B, J)
    c2 = np.asarray(conv2, dtype=np.float32).reshape(B, J)
    c1_bf = c1.astype(ml_dtypes.bfloat16)
    c2_bf = c2.astype(ml_dtypes.bfloat16)

    # blockdiag[p, m] = 1/J if p//RPS == m//RPS else 0
    bd = (
        np.kron(np.eye(BPC, dtype=np.float32), np.ones((RPS, RPS), dtype=np.float32))
        / np.float32(J)
    ).astype(np.float32)

    in_maps = []
    for i in range(NCORES):
        sl = slice(i * BPC, (i + 1) * BPC)
        in_maps.append(
            {
                "conv1": np.ascontiguousarray(c1_bf[sl].reshape(P, F)),
                "conv2": np.ascontiguousarray(c2_bf[sl].reshape(P, F)),
                "blockdiag": bd,
            }
        )

    nc = _build_nc()
    res = run_bass_kernel_spmd(nc, in_maps, list(range(NCORES)), trace=bool(_trace))
    LAST_RESULTS = res
    out = np.concatenate(
        [
            np.asarray(res.results[i]["out"]).reshape(BPC, J)
            for i in range(NCORES)
        ],
        axis=0,
    ).astype(np.float32)
    return out
